# revision 17
# baseline (speedup 1.0000x reference)
"""Trainium2 Bass kernel for nn_Attention_interaction (dense_transformer).

Math (per batch b, head h):
    q = l2norm(x);  S = (q @ q^T) / SCALE / attn_gamma;  P = softmax(S, -1)
    o = P @ y;  o2 = o @ W^T + bias;  out = w0*y + w1*o2
with w_i = exp(sum_gamma_i) / (exp(sum_gamma0) + exp(sum_gamma1)).

Sharding: batch dim B=8 across the 8 cores (1 batch x 8 heads per core).

Design notes (measured on HW, not the cost model):
  - K<=64 matmuls stream at 2 cycles/output-column on TRN2; K>=65 at 1.
    So S = q q^T runs as zero-padded K=128 bf16 matmuls: per head a
    [128,1024] operand tile holds the 64 q rows on partitions 0-63 (head A)
    or 64-127 (head B) with the other half zeroed once; the stationary's
    zero half masks the pair partner. This halves S time vs K=64/DoubleRow.
  - exp splits between ACT (native Exp -> fp8 E) and DVE (Schraudolph on
    fp8e4 bit patterns: bits8 = round(s*8/ln2 + (56 - 0.375)), ~7% max rel
    err, inside the 2e-2 gate), balanced by a greedy time counter.
  - Heads run sequentially, software-pipelined: each head's O-flush/evac/
    proj/epilogue tail is deferred into the next head's chunk stream so the
    exp engines never drain. PSUM = 3 S-chunk slots + one [128,1024] O tile
    per head (proj pj reuses the evacuated jc-half of the same tile).
  - O^T = ya^T E uses E symmetry; softmax denominators ride in the O
    matmul's 65th row (ones column in ya, fp8); r returns via a DRAM bounce
    as [128,8] for one reciprocal; epilogue o2 = pj * rinv uses a stride-0
    broadcast AP; fin = o2 + w0*y on GpSimd; q-norm squares on GpSimd,
    reduces/Newton-rsqrt/scales/masked copies on DVE.
  - A dummy scalar-engine memzero at the top pulls the ACT table load into
    the preamble so the q^T DMA-xbar transposes on the ACT queue start
    immediately.
"""

import math

import numpy as np
import ml_dtypes

import concourse.bass as bass
import concourse.bacc as bacc
import concourse.tile as tile
from concourse import mybir
from concourse.bass_utils import run_bass_kernel_spmd
from concourse._compat import get_trn_type

B, H, N, D = 8, 8, 1024, 64
SCALE = (512 // 8) ** (-0.5)  # 0.125
EPS = 1e-6
NCORES = 8
NB = N // 128  # 8 row blocks of 128
NW = N * NB  # 8192 flattened S columns per head
CHUNK = 1024
F32 = mybir.dt.float32
BF16 = mybir.dt.bfloat16
FP8 = mybir.dt.float8e4
I16 = mybir.dt.int16
U8 = mybir.dt.uint8
I32 = mybir.dt.int32
AX = mybir.AxisListType
OP = mybir.AluOpType
ACT = mybir.ActivationFunctionType
PM = mybir.MatmulPerfMode
MAGIC = 0x5F3759DF

# Schraudolph exp on fp8e4 bit patterns: bits = round(s*A_SCH + B_SCH)
A_SCH = 8.0 / math.log(2.0)
SIGMA = 0.375
B_SCH = 7.0 * 8.0 - SIGMA

# greedy exp-engine balancing: estimated op costs in us
ACT_CHUNK = 1.111
DVE_CHUNK = 1.218
ACT_EVAC = 0.69
DVE_EVAC = 0.69
DVE_O2 = 0.78
DVE_RECIP = 0.30

LAST_RESULTS = None  # BassKernelResults of the most recent run (for test.py)


def _emit(ctx, tc, sqrt_c2: float):
    nc = tc.nc
    xq = nc.dram_tensor("xq", [H, N, D], BF16, kind="ExternalInput")
    ya = nc.dram_tensor("ya", [H, N, D + 1], FP8, kind="ExternalInput")
    yb = nc.dram_tensor("yb", [H, N, D], F32, kind="ExternalInput")
    wt = nc.dram_tensor("wt", [D + 1, D], BF16, kind="ExternalInput")
    out = nc.dram_tensor("out", [H, N, D], F32, kind="ExternalOutput")
    rscr = nc.dram_tensor("rscr", [H, N], BF16)  # denominator bounce

    singles = ctx.enter_context(tc.tile_pool(name="singles", bufs=1))
    io = ctx.enter_context(tc.tile_pool(name="io", bufs=3))
    st = ctx.enter_context(tc.tile_pool(name="st", bufs=2))
    qpool = ctx.enter_context(tc.tile_pool(name="qpool", bufs=1))
    epool = ctx.enter_context(tc.tile_pool(name="epool", bufs=2))
    wpool = ctx.enter_context(tc.tile_pool(name="wpool", bufs=2))
    ps_s = ctx.enter_context(tc.tile_pool(name="ps_s", bufs=3, space="PSUM"))
    ps_o = ctx.enter_context(tc.tile_pool(name="ps_o", bufs=1, space="PSUM"))

    qTz = {}  # per head: [128, 1024] bf16, rows 0-63|64-127 = d, rest zero
    eng_t = {"act": 0.0, "dve": 0.0}  # greedy engine-time counters

    def qprep(p):
        """q-prep for pair p (heads 2p, 2p+1): load, norms, scale to bf16 q,
        packed DMA-xbar transposes, then masked copies into zero-padded
        K=128 operand tiles (head A on partitions 0-63, B on 64-127)."""
        hA, hB = 2 * p, 2 * p + 1
        x2 = io.tile([128, 2, NB, D], BF16, tag="x2", name=f"x2_{p}")
        nc.sync.dma_start(
            out=x2,
            in_=xq[hA : hA + 2].rearrange("h (b p) d -> p h b d", p=128),
        )
        ss = st.tile([128, 2, NB], F32, tag="ssp", name=f"ss{p}")
        sqeng = nc.vector if p == 0 else nc.gpsimd
        for hi in range(2):
            sq = st.tile([128, NB, D], BF16, tag="sqp", name=f"sq{p}")
            sqeng.tensor_mul(sq, x2[:, hi], x2[:, hi])
            nc.vector.tensor_reduce(ss[:, hi], sq, axis=AX.X, op=OP.add)

        # rs = sqrt_c2 / sqrt(ss + eps): fast inverse sqrt + 1 Newton (DVE)
        ssf = ss.rearrange("p h b -> p (h b)")
        half = st.tile([128, 16], F32, tag="halfp", name=f"half{p}")
        nc.vector.tensor_scalar(
            out=half, in0=ssf, scalar1=0.5, scalar2=0.5 * EPS,
            op0=OP.mult, op1=OP.add,
        )
        rs = st.tile([128, 16, 1], F32, tag="rsp", name=f"rs{p}")
        rsf = rs.rearrange("p a one -> p (a one)")
        yi = rsf.bitcast(I32)
        nc.vector.tensor_scalar(
            out=yi, in0=ssf.bitcast(I32), scalar1=1, scalar2=None,
            op0=OP.logical_shift_right,
        )
        nc.vector.tensor_scalar(
            out=yi, in0=yi, scalar1=MAGIC, scalar2=-1,
            op0=OP.subtract, op1=OP.mult,
        )
        t1 = st.tile([128, 16], F32, tag="t1p", name=f"t1{p}")
        nc.vector.tensor_mul(t1, rsf, rsf)
        nc.vector.tensor_mul(t1, t1, half)
        nc.vector.tensor_scalar(
            out=t1, in0=t1, scalar1=1.5, scalar2=-sqrt_c2,
            op0=OP.subtract, op1=OP.mult,
        )
        nc.vector.tensor_mul(rsf, rsf, t1)

        # q (bf16) packed [A|B] per block, transpose per block, then masked
        # copies into the zero-padded operand tiles.
        q4 = qpool.tile([128, NB, 128], BF16, tag="q4p", name=f"q4_{p}")
        qTp = qpool.tile([128, NB, 128], BF16, tag="qTp", name=f"qTp_{p}")
        for hi in range(2):
            nc.vector.tensor_mul(
                q4[:, :, hi * 64 : (hi + 1) * 64],
                x2[:, hi],
                rs[:, hi * NB : (hi + 1) * NB, :].broadcast_to([128, NB, D]),
            )
        za = qpool.tile([128, N], BF16, tag="qza", name=f"qza{p}")
        zb = qpool.tile([128, N], BF16, tag="qzb", name=f"qzb{p}")
        if p < 2:  # first use of each physical buffer: zero the pad halves
            nc.gpsimd.memset(za[64:128, :], 0.0)
            nc.gpsimd.memset(zb[0:64, :], 0.0)
        qf = qTp.rearrange("p b t -> p (b t)")
        for b in range(NB):
            dq = nc.scalar if b % 2 == 1 else nc.sync
            dq.dma_start(out=qTp[:, b, :], in_=q4[:, b, :], transpose=True)
            if b % 4 == 3:
                b0, b1 = (b - 3) * 128, (b + 1) * 128
                nc.vector.tensor_copy(za[0:64, b0:b1], qf[0:64, b0:b1])
                nc.vector.tensor_copy(zb[64:128, b0:b1], qf[64:128, b0:b1])
        qTz[hA], qTz[hB] = za, zb

    def q_lhsT(h, i):
        return qTz[h][:, i * 128 : (i + 1) * 128]

    def q_rhs(h, jc):
        return qTz[h][:, jc * 512 : (jc + 1) * 512]

    wt_sb = singles.tile([D + 1, D], BF16)
    dummy = singles.tile([1, 4], F32, name="dummy")
    nc.scalar.memzero(dummy)  # pull the ACT table load into the preamble

    qprep(0)
    nc.sync.dma_start(out=wt_sb, in_=wt[:, :])

    def load_head(h):
        ya_t = io.tile([128, NB, D + 1], FP8, tag="ya", name=f"ya{h}")
        nc.sync.dma_start(
            out=ya_t, in_=ya[h].rearrange("(b p) d -> p b d", p=128)
        )
        yb_t = io.tile([128, NB, D], F32, tag="yb", name=f"yb{h}")
        nc.sync.dma_start(
            out=yb_t, in_=yb[h].rearrange("(b p) d -> p b d", p=128)
        )
        return ya_t, yb_t

    pend = load_head(0)

    class Head:
        pass

    def make_head(h):
        st_ = Head()
        st_.h = h
        st_.ya_t, st_.yb_t = pend
        st_.E = epool.tile([128, NW], FP8, tag="E", name=f"E{h}")
        st_.Ei = st_.E.bitcast(U8)
        st_.po = ps_o.tile([128, 1024], F32, tag="o", name=f"o{h}")
        st_.OT = wpool.tile([D + 1, N], BF16, tag="OT", name=f"OT{h}")
        st_.ok = 0
        return st_

    def emit_o(hs, limit):
        while hs.ok < 16:
            k = hs.ok
            jc, i = k // NB, k % NB
            if jc * 4096 + (i + 1) * 512 > limit:
                return
            nc.tensor.matmul(
                hs.po[0 : D + 1, jc * 512 : (jc + 1) * 512],
                lhsT=hs.ya_t[:, i, :],
                rhs=hs.E[:, jc * 4096 + i * 512 : jc * 4096 + (i + 1) * 512],
                start=(i == 0), stop=(i == NB - 1), tile_position=(0, 0),
            )
            hs.ok += 1

    def evac(hs, jc):
        src_ = hs.po[0 : D + 1, jc * 512 : (jc + 1) * 512]
        dst = hs.OT[:, jc * 512 : (jc + 1) * 512]
        if eng_t["act"] + ACT_EVAC <= eng_t["dve"] + DVE_EVAC:
            nc.scalar.copy(out=dst, in_=src_)
            eng_t["act"] += ACT_EVAC
        else:
            nc.vector.tensor_copy(dst, src_)
            eng_t["dve"] += DVE_EVAC

    def emit_proj(hs, jc):
        for bb in range(4):
            b = jc * 4 + bb
            nc.tensor.matmul(
                hs.po[:, jc * 512 + bb * 64 : jc * 512 + (bb + 1) * 64],
                lhsT=hs.OT[:, b * 128 : (b + 1) * 128],
                rhs=wt_sb,
                start=True, stop=True, tile_position=(0, 0),
            )

    def tail_a(hs):
        # O flush for jc1
        emit_o(hs, NW)

    def tail_b(hs):
        evac(hs, 1)
        nc.sync.dma_start(out=rscr[hs.h], in_=hs.OT[D : D + 1, :])
        hs.rT = st.tile([128, NB], BF16, tag="rT", name=f"rT{hs.h}")
        nc.sync.dma_start(
            out=hs.rT, in_=rscr[hs.h].rearrange("(b p) -> p b", p=128)
        )
        emit_proj(hs, 1)

    def tail_c(hs):
        rinv = st.tile([128, NB, 1], F32, tag="rinv", name=f"rinv{hs.h}")
        nc.vector.reciprocal(rinv.rearrange("p a one -> p (a one)"), hs.rT)
        eng_t["dve"] += DVE_RECIP
        pj = (
            hs.po.rearrange("p (jc x) -> p jc x", jc=2)[:, :, 0:256]
            .rearrange("p jc (bb d) -> p jc bb d", bb=4)
        )
        rb = (
            rinv.rearrange("p (jc bb) one -> p jc bb one", jc=2)
            .broadcast_to([128, 2, 4, D])
        )
        o2 = wpool.tile([128, NB, D], F32, tag="o2", name=f"o2{hs.h}")
        nc.vector.tensor_mul(
            o2.rearrange("p (jc bb) d -> p jc bb d", jc=2), pj, rb
        )
        eng_t["dve"] += DVE_O2
        fin = wpool.tile([128, NB, D], F32, tag="fin", name=f"fin{hs.h}")
        if hs.h >= H - 2:
            nc.vector.tensor_add(fin, o2, hs.yb_t)
            eng_t["dve"] += 0.66
        else:
            nc.gpsimd.tensor_add(fin, o2, hs.yb_t)
        nc.sync.dma_start(
            out=out[hs.h].rearrange("(b p) d -> p b d", p=128), in_=fin
        )

    prev = None
    for h in range(H):
        hs = make_head(h)
        for c in range(NB):
            # deferred tail of the previous head, interleaved with our chunks
            if prev is not None:
                if c == 1:
                    tail_a(prev)
                elif c == 2:
                    tail_b(prev)
                elif c == 4:
                    tail_c(prev)
                    prev = None
            jc, ip = c // 4, (c % 4) * 2
            ps = ps_s.tile([128, CHUNK], F32, tag="psS", name="psS")
            for k in range(2):
                nc.tensor.matmul(
                    ps[:, k * 512 : (k + 1) * 512],
                    lhsT=q_lhsT(hs.h, ip + k),
                    rhs=q_rhs(hs.h, jc),
                    start=True, stop=True, tile_position=(0, 0),
                )
            if eng_t["dve"] + DVE_CHUNK < eng_t["act"] + ACT_CHUNK:
                nc.vector.tensor_scalar(
                    out=hs.Ei[:, c * CHUNK : (c + 1) * CHUNK],
                    in0=ps, scalar1=A_SCH, scalar2=B_SCH,
                    op0=OP.mult, op1=OP.add,
                )
                eng_t["dve"] += DVE_CHUNK
            else:
                nc.scalar.activation(
                    out=hs.E[:, c * CHUNK : (c + 1) * CHUNK], in_=ps,
                    func=ACT.Exp,
                )
                eng_t["act"] += ACT_CHUNK
            emit_o(hs, (c - 1) * CHUNK)
            if c == 0:
                if h + 1 < H:
                    pend = load_head(h + 1)
                if h % 2 == 1 and h + 1 < H:
                    qprep((h + 1) // 2)
            elif c == 6:
                evac(hs, 0)
            elif c == 7:
                emit_proj(hs, 0)
        prev = hs

    tail_a(prev)
    tail_b(prev)
    tail_c(prev)


def build_program(sqrt_c2: float) -> bass.Bass:
    from contextlib import ExitStack

    nc = bacc.Bacc(get_trn_type() or "TRN2", target_bir_lowering=False)
    with tile.TileContext(nc) as tc:
        with ExitStack() as ctx:
            _emit(ctx, tc, sqrt_c2)
    nc.compile()
    return nc


def make_inputs(x, y, proj_w, proj_b, attn_gamma, sum_gamma0, sum_gamma1):
    x = np.asarray(x, dtype=np.float32)
    y = np.asarray(y, dtype=np.float32)
    proj_w = np.asarray(proj_w, dtype=np.float32)
    proj_b = np.asarray(proj_b, dtype=np.float32)
    g0 = math.exp(float(np.asarray(sum_gamma0)))
    g1 = math.exp(float(np.asarray(sum_gamma1)))
    w0 = g0 / (g0 + g1)
    w1 = g1 / (g0 + g1)
    c2 = 1.0 / (SCALE * float(np.asarray(attn_gamma)))

    xq = x.astype(ml_dtypes.bfloat16)
    yac = np.concatenate(
        [y, np.ones(y.shape[:-1] + (1,), np.float32)], axis=-1
    ).astype(ml_dtypes.float8_e4m3fn)
    ybv = (w0 * y).astype(np.float32)
    wtv = np.concatenate([proj_w.T * w1, w1 * proj_b[None, :]], axis=0).astype(
        ml_dtypes.bfloat16
    )
    in_maps = [
        {"xq": xq[c], "ya": yac[c], "yb": ybv[c], "wt": wtv}
        for c in range(NCORES)
    ]
    return in_maps, math.sqrt(c2)


def kernel(x, y, proj_w, proj_b, attn_gamma, sum_gamma0, sum_gamma1):
    global LAST_RESULTS
    in_maps, sqrt_c2 = make_inputs(
        x, y, proj_w, proj_b, attn_gamma, sum_gamma0, sum_gamma1
    )
    nc = build_program(sqrt_c2)
    res = run_bass_kernel_spmd(nc, in_maps, list(range(NCORES)))
    LAST_RESULTS = res
    return np.stack([res.results[c]["out"] for c in range(NCORES)], axis=0)
